# revision 2
# baseline (speedup 1.0000x reference)
"""GCN (2-layer GCNConv + mean readout + sigmoid head) on 8 Trainium2 NeuronCores.

Strategy (graph/data parallel, dst-sharded):
  - Nodes are permuted (round-robin by in-degree) into NB = n_cores*nblk blocks of
    128 so every block has ~equal in-edge count; each core owns nblk blocks.
  - Per layer: H' = (D^-1/2 Z) @ W computed node-sharded on PE (bf16), written to
    an fp8 feature table, AllGather (Shared output) of the fp8 table, then per
    dst-block: dma_gather of the 256B source rows (table split in two halves so
    row ids fit int16), host-precomputed fp8 one-hot selection matrices streamed
    from HBM via HWDGE, contracted on PE to form segment sums in PSUM.
    Self-loops are added via an fp8 identity-matrix matmul on the local shard.
  - The wall-clock limiter is SWDGE descriptor generation (~7.6 ns/idx on the
    4 Q7 queue pairs), so the gathers are organized to keep all 4 queues busy:
    2 dst-blocks per gather, lo gathers lead hi gathers by LEAD groups in the
    Pool stream (the hi gather's AllGather wait would otherwise head-of-line
    block later lo desc-gens), and deep msg buffering.
  - Layer-2 work is fused into layer-1 aggregation: each z1 block is
    PE-transposed (identity matmul) into zT, immediately pushed through the
    layer-2 dense matmul, and the layer-2 AllGathers fire mid-loop (lo as soon
    as the lo half of h2 exists) so they hide under remaining aggregation.
  - A tiny dummy AllGather issued at t~0 absorbs the collectives entry barrier.
  - dinv factors are separable: dinv_src is folded into the table rows,
    dinv_dst + relu are applied post-aggregation in ONE ScalarE activation.
  - Readout: per-block column sums via matmul against a pad-mask vector,
    accumulated in PSUM; final cross-core reduce + fc + sigmoid on host.
"""

import math

import numpy as np
import ml_dtypes

BF16 = ml_dtypes.bfloat16
FP8 = ml_dtypes.float8_e4m3  # == mybir.dt.float8e4 (IEEE, max +-240)

# Problem constants (hardcoded per contract; kernel.py must be self-contained).
N = 50000
E = 800000
IN_DIM = 512
HID = 256
N_CORES = 8
P = 128
GBV = 2   # dst-blocks per dma_gather instruction
LEAD = 6  # lo gathers lead hi gathers by this many groups in the Pool stream


def _wrap_idx(flat):
    """[L] int -> [128, L/16] int16 in the SWDGE wrapped layout."""
    L = len(flat)
    assert L % 16 == 0
    w = flat.reshape(L // 16, 16).T  # value i at [i%16, i//16]
    return np.ascontiguousarray(np.tile(w, (8, 1)).astype(np.int16))


# --------------------------------------------------------------------------- #
# Host-side preprocessing
# --------------------------------------------------------------------------- #

def _preprocess(x, edge_index, W1, b1, W2, b2):
    n, in_dim = x.shape
    hid = W1.shape[1]
    src = np.asarray(edge_index[0], dtype=np.int64)
    dst = np.asarray(edge_index[1], dtype=np.int64)

    deg_in = np.bincount(dst, minlength=n)
    deg = deg_in.astype(np.float64) + 1.0  # + self loop
    dinv = (1.0 / np.sqrt(deg)).astype(np.float32)

    nblk = math.ceil(n / (N_CORES * P))          # blocks per core
    NB = N_CORES * nblk                          # total blocks
    npad = NB * P
    nshard = nblk * P
    nlo_b = (nblk + 1) // 2                      # lo blocks per core
    nhi_b = nblk - nlo_b
    nlo, nhi = nlo_b * P, nhi_b * P
    assert N_CORES * nlo <= 32767, "lo table must fit int16"

    # Balance blocks: deal nodes round-robin across blocks in desc in-degree
    # order -> every block gets ~equal total in-degree.
    order = np.argsort(-deg_in, kind="stable")
    i = np.arange(n)
    new_id = np.empty(n, dtype=np.int64)
    new_id[order] = (i % NB) * P + (i // NB)

    # Edge arrays in permuted space, sorted by (dst block, src half).
    # Source "half" = within-shard block position (lo = first nlo_b blocks of
    # each core's shard), so the lo table is complete after the first
    # AllGather chunk and lo gathers can overlap with the second chunk.
    s_new = new_id[src]
    d_new = new_id[dst]
    blk_id = d_new // P
    s_core = (s_new // P) // nblk
    s_blocal = (s_new // P) % nblk
    s_pos = s_blocal * P + (s_new % P)
    is_hi = (s_blocal >= nlo_b).astype(np.int64)
    s_lo_id = s_core * nlo + s_pos
    s_hi_id = s_core * nhi + (s_pos - nlo)
    skey = blk_id * 2 + is_hi
    eorder = np.argsort(skey, kind="stable")
    s_lo_id = s_lo_id[eorder]
    s_hi_id = s_hi_id[eorder]
    d_new = d_new[eorder]
    key_sorted = skey[eorder]

    cnt = np.bincount(key_sorted, minlength=2 * NB).reshape(NB, 2)
    c_lo = max(1, int(math.ceil(cnt[:, 0].max() / P)))
    c_hi = max(1, int(math.ceil(cnt[:, 1].max() / P)))
    c_tot = c_lo + c_hi

    # Per-(block, half) padded slots.
    idx_lo = np.zeros((NB, c_lo * P), dtype=np.int64)
    idx_hi = np.zeros((NB, c_hi * P), dtype=np.int64)
    dst_arr = np.full((NB, c_tot * P), -1.0, dtype=np.float32)

    starts = np.zeros(2 * NB + 1, dtype=np.int64)
    np.cumsum(cnt.reshape(-1), out=starts[1:])
    pos = np.arange(len(s_new)) - starts[key_sorted]
    lo_m = key_sorted % 2 == 0
    hi_m = ~lo_m
    b_lo, b_hi = key_sorted[lo_m] // 2, key_sorted[hi_m] // 2
    idx_lo[b_lo, pos[lo_m]] = s_lo_id[lo_m]
    idx_hi[b_hi, pos[hi_m]] = s_hi_id[hi_m]
    dst_arr[b_lo, pos[lo_m]] = (d_new[lo_m] % P).astype(np.float32)
    dst_arr[b_hi, c_lo * P + pos[hi_m]] = (d_new[hi_m] % P).astype(np.float32)

    dst_arr = dst_arr.reshape(NB, c_tot, P)

    # Host-built one-hot scatter matrices: st[eslot, (blk, c, dstq)] in fp8.
    # (dst_arr == -1 padding rows become all-zero one-hot rows.)
    onehot = (dst_arr[:, :, :, None] ==
              np.arange(P, dtype=np.float32)[None, None, None, :])
    # [NB, c_tot, eslot, dstq] -> [eslot, NB, c_tot, dstq]
    st_all = np.ascontiguousarray(
        onehot.transpose(2, 0, 1, 3).astype(FP8))

    # x' = dinv * x, permuted, padded, per-core transposed, bf16.
    xp = np.zeros((npad, in_dim), dtype=np.float32)
    xp[new_id] = x * dinv[:, None]

    dinv_pad = np.zeros(npad, dtype=np.float32)
    dinv_pad[new_id] = dinv
    inv_dinv_pad = np.zeros(npad, dtype=np.float32)
    inv_dinv_pad[new_id] = 1.0 / dinv
    mask_pad = np.zeros(npad, dtype=np.float32)
    mask_pad[new_id] = 1.0

    ident = np.eye(P, dtype=np.float32)
    has_b1 = bool(np.any(np.asarray(b1) != 0.0))
    has_b2 = bool(np.any(np.asarray(b2) != 0.0))

    common = {
        "w1": np.ascontiguousarray(W1.astype(BF16)),
        "w2": np.ascontiguousarray(W2.astype(BF16)),
        "brow1": np.ascontiguousarray(np.asarray(b1, np.float32).reshape(1, hid)),
        "brow2": np.ascontiguousarray(np.asarray(b2, np.float32).reshape(1, hid)),
        "ident": np.ascontiguousarray(ident.astype(FP8)),
        "identbf": np.ascontiguousarray(ident.astype(BF16)),
    }

    in_maps = []
    for c in range(N_CORES):
        lo_b, hi_b = c * nblk, (c + 1) * nblk
        lo_n, hi_n = c * nshard, (c + 1) * nshard
        m = dict(common)
        m["xT"] = np.ascontiguousarray(xp[lo_n:hi_n].T.astype(BF16))
        m["idxlo"] = _wrap_idx(idx_lo[lo_b:hi_b].reshape(-1))
        m["idxhi"] = _wrap_idx(idx_hi[lo_b:hi_b].reshape(-1))
        # [eslot, nblk, c_tot, dstq] -> [P, nblk*c_tot*P]
        m["sth"] = np.ascontiguousarray(
            st_all[:, lo_b:hi_b].reshape(P, nblk * c_tot * P))
        m["dinv"] = np.ascontiguousarray(
            dinv_pad[lo_n:hi_n].reshape(nblk, P).T.astype(np.float32))
        m["idinv"] = np.ascontiguousarray(
            inv_dinv_pad[lo_n:hi_n].reshape(1, nshard).astype(np.float32))
        m["maskc"] = np.ascontiguousarray(
            mask_pad[lo_n:hi_n].reshape(nblk, P).T.astype(BF16))
        in_maps.append(m)

    meta = dict(nblk=nblk, nlo_b=nlo_b, c_lo=c_lo, c_hi=c_hi, in_dim=in_dim,
                hid=hid, n=n, has_b1=has_b1, has_b2=has_b2)
    return in_maps, meta


# --------------------------------------------------------------------------- #
# Device program
# --------------------------------------------------------------------------- #

def _build_nc(nblk, nlo_b, c_lo, c_hi, in_dim, hid, has_b1=False, has_b2=False):
    from contextlib import ExitStack

    from concourse import bass, mybir, bacc
    import concourse.tile as tile

    dt = mybir.dt
    nshard = nblk * P
    npad = N_CORES * nshard
    nhi_b = nblk - nlo_b
    nlo, nhi = nlo_b * P, nhi_b * P
    KIN = in_dim // P
    KH = hid // P
    c_tot = c_lo + c_hi
    ngrp = math.ceil(nblk / GBV)
    # group after which all lo blocks' layer-2 dense outputs exist
    g_lo2 = math.ceil(nlo_b / GBV) - 1

    nc = bacc.Bacc(None, target_bir_lowering=False, num_devices=N_CORES,
                   num_swdge_queues=4)

    xT = nc.dram_tensor("xT", [in_dim, nshard], dt.bfloat16, kind="ExternalInput")
    w1 = nc.dram_tensor("w1", [in_dim, hid], dt.bfloat16, kind="ExternalInput")
    w2 = nc.dram_tensor("w2", [hid, hid], dt.bfloat16, kind="ExternalInput")
    brow1 = nc.dram_tensor("brow1", [1, hid], dt.float32, kind="ExternalInput")
    brow2 = nc.dram_tensor("brow2", [1, hid], dt.float32, kind="ExternalInput")
    ident = nc.dram_tensor("ident", [P, P], dt.float8e4, kind="ExternalInput")
    identbf = nc.dram_tensor("identbf", [P, P], dt.bfloat16, kind="ExternalInput")
    idxlo = nc.dram_tensor("idxlo", [P, nblk * c_lo * 8], dt.int16, kind="ExternalInput")
    idxhi = nc.dram_tensor("idxhi", [P, nblk * c_hi * 8], dt.int16, kind="ExternalInput")
    sth = nc.dram_tensor("sth", [P, nblk * c_tot * P], dt.float8e4, kind="ExternalInput")
    dinv = nc.dram_tensor("dinv", [P, nblk], dt.float32, kind="ExternalInput")
    idinv = nc.dram_tensor("idinv", [1, nshard], dt.float32, kind="ExternalInput")
    maskc = nc.dram_tensor("maskc", [P, nblk], dt.bfloat16, kind="ExternalInput")
    out = nc.dram_tensor("partial", [P, KH], dt.float32, kind="ExternalOutput")

    with tile.TileContext(nc) as tc, ExitStack() as ctx:
        const = ctx.enter_context(tc.tile_pool(name="const", bufs=1))
        persist = ctx.enter_context(tc.tile_pool(name="persist", bufs=1))
        lhsp = ctx.enter_context(tc.tile_pool(name="lhsp", bufs=1))
        mlop = ctx.enter_context(tc.tile_pool(name="mlop", bufs=LEAD + 2))
        mhip = ctx.enter_context(tc.tile_pool(name="mhip", bufs=3))
        stp = ctx.enter_context(tc.tile_pool(name="stp", bufs=3))
        zp = ctx.enter_context(tc.tile_pool(name="zp", bufs=4))
        ps_mm = ctx.enter_context(tc.tile_pool(name="ps_mm", bufs=2, space="PSUM"))
        ps_agg = ctx.enter_context(tc.tile_pool(name="ps_agg", bufs=2, space="PSUM"))
        ps_tr = ctx.enter_context(tc.tile_pool(name="ps_tr", bufs=2, space="PSUM"))
        ps_cs = ctx.enter_context(tc.tile_pool(name="ps_cs", bufs=1, space="PSUM"))
        dram = ctx.enter_context(tc.tile_pool(name="dram", bufs=1, space="DRAM"))

        # ---- persistent / constant tiles ----
        w1_sb = const.tile([P, KIN * hid], dt.bfloat16, tag="w1_sb")
        w2_sb = const.tile([P, KH * hid], dt.bfloat16, tag="w2_sb")
        brow1_sb = const.tile([1, hid], dt.float32, tag="brow1_sb")
        brow2_sb = const.tile([1, hid], dt.float32, tag="brow2_sb")
        ident_sb = const.tile([P, P], dt.float8e4, tag="ident_sb")
        identbf_sb = const.tile([P, P], dt.bfloat16, tag="identbf_sb")
        idxlo_sb = const.tile([P, nblk * c_lo * 8], dt.int16, tag="idxlo_sb")
        idxhi_sb = const.tile([P, nblk * c_hi * 8], dt.int16, tag="idxhi_sb")
        dinv_sb = const.tile([P, nblk], dt.float32, tag="dinv_sb")
        mask_sb = const.tile([P, nblk], dt.bfloat16, tag="mask_sb")
        if has_b1 or has_b2:
            idinv_sb = const.tile([1, nshard], dt.float32, tag="idinv_sb")

        zT_sb = persist.tile([P, KH * nshard], dt.bfloat16, tag="zT_sb")
        h_sb = persist.tile([P, nblk * hid], dt.float8e4, tag="h_sb")

        hshard_lo_d = dram.tile([nlo, hid], dt.float8e4, tag="hshard_lo_d")
        hshard_hi_d = dram.tile([nhi, hid], dt.float8e4, tag="hshard_hi_d")
        dummy_d = dram.tile([1, 4], dt.float32, tag="dummy_d")
        # Shared DRAM (collective outputs) may be written by a single inst
        # each -> one table per (layer, half).
        tlo1_d = dram.tile([N_CORES * nlo, hid], dt.float8e4, tag="tlo1_d",
                           addr_space="Shared")
        thi1_d = dram.tile([N_CORES * nhi, hid], dt.float8e4, tag="thi1_d",
                           addr_space="Shared")
        tlo2_d = dram.tile([N_CORES * nlo, hid], dt.float8e4, tag="tlo2_d",
                           addr_space="Shared")
        thi2_d = dram.tile([N_CORES * nhi, hid], dt.float8e4, tag="thi2_d",
                           addr_space="Shared")
        tdummy_d = dram.tile([N_CORES, 4], dt.float32, tag="tdummy_d",
                             addr_space="Shared")

        # ---- constant loads (w1 + x first: they gate dense1) ----
        nc.sync.dma_start(
            out=w1_sb[:, :].rearrange("p (k f) -> p k f", k=KIN),
            in_=w1[:, :].rearrange("(k p) f -> p k f", p=P))
        xfull = lhsp.tile([P, KIN * nshard], dt.bfloat16, tag="xfull")
        # split by node range (lo first) so dense1-lo can start early
        nc.sync.dma_start(
            out=xfull[:, :].rearrange("p (k n) -> p k n", k=KIN)[:, :, :nlo],
            in_=xT[:, :nlo].rearrange("(k p) n -> p k n", p=P))
        nc.sync.dma_start(
            out=xfull[:, :].rearrange("p (k n) -> p k n", k=KIN)[:, :, nlo:],
            in_=xT[:, nlo:].rearrange("(k p) n -> p k n", p=P))
        nc.sync.dma_start(
            out=w2_sb[:, :].rearrange("p (k f) -> p k f", k=KH),
            in_=w2[:, :].rearrange("(k p) f -> p k f", p=P))
        nc.sync.dma_start(out=brow1_sb[:, :], in_=brow1[:, :])
        nc.sync.dma_start(out=brow2_sb[:, :], in_=brow2[:, :])
        nc.sync.dma_start(out=ident_sb[:, :], in_=ident[:, :])
        nc.sync.dma_start(out=identbf_sb[:, :], in_=identbf[:, :])
        nc.sync.dma_start(out=idxlo_sb[:, :], in_=idxlo[:, :])
        nc.sync.dma_start(out=idxhi_sb[:, :], in_=idxhi[:, :])
        nc.sync.dma_start(out=dinv_sb[:, :], in_=dinv[:, :])
        nc.sync.dma_start(out=mask_sb[:, :], in_=maskc[:, :])
        if has_b1 or has_b2:
            nc.sync.dma_start(out=idinv_sb[:, :], in_=idinv[:, :])

        # ---- dummy collective: absorbs the cc entry barrier at t~0 ----
        nc.scalar.dma_start(out=dummy_d[:, :], in_=brow1_sb[0:1, 0:4])
        nc.gpsimd.collective_compute(
            "AllGather", mybir.AluOpType.bypass,
            replica_groups=[list(range(N_CORES))],
            ins=[dummy_d[:, :].opt()],
            outs=[tdummy_d[:, :].opt()])

        def dense1(nb):
            """h_sb[:, nb*hid:...] = fp8(x' @ W1) for one block."""
            ps = ps_mm.tile([P, hid], dt.float32, tag="mm")
            for k in range(KIN):
                nc.tensor.matmul(
                    out=ps[:, :],
                    lhsT=xfull[:, k * nshard + nb * P:k * nshard + (nb + 1) * P],
                    rhs=w1_sb[:, k * hid:(k + 1) * hid],
                    start=(k == 0), stop=(k == KIN - 1))
            nc.scalar.activation(
                h_sb[:, nb * hid:(nb + 1) * hid], ps[:, :],
                mybir.ActivationFunctionType.Copy)

        def distribute_lo(table_d):
            nc.scalar.dma_start(
                out=hshard_lo_d[:, :].rearrange("(nb p) f -> p nb f", p=P),
                in_=h_sb[:, :nlo_b * hid].rearrange("p (nb f) -> p nb f",
                                                    nb=nlo_b))
            nc.gpsimd.collective_compute(
                "AllGather", mybir.AluOpType.bypass,
                replica_groups=[list(range(N_CORES))],
                ins=[hshard_lo_d[:, :].opt()],
                outs=[table_d[:, :].opt()])

        def distribute_hi_dma():
            nc.scalar.dma_start(
                out=hshard_hi_d[:, :].rearrange("(nb p) f -> p nb f", p=P),
                in_=h_sb[:, nlo_b * hid:].rearrange("p (nb f) -> p nb f",
                                                    nb=nhi_b))

        def distribute_hi_cc(table_d):
            nc.gpsimd.collective_compute(
                "AllGather", mybir.AluOpType.bypass,
                replica_groups=[list(range(N_CORES))],
                ins=[hshard_hi_d[:, :].opt()],
                outs=[table_d[:, :].opt()])

        def aggregate(tlo_d, thi_d, brow_sb, has_b, z_consumer,
                      pre_loop=None, post_group=None):
            """Gather + one-hot contract per dst block; z_consumer(nb, z).

            Pool-stream order: lo gathers lead hi gathers by LEAD groups so a
            hi gather waiting on the hi AllGather never head-of-line blocks
            lo descriptor generation. pre_loop() is emitted after the lo
            prefix (used to trigger the NEXT table's hi AllGather without
            blocking this layer's lo desc-gen).
            """
            qi = [0]

            def next_q():
                q = qi[0] % 4
                qi[0] += 1
                return q

            st_t, mlo_t, mhi_t = {}, {}, {}

            def emit_lo(g):
                g0 = g * GBV
                gb = min(GBV, nblk - g0)
                mlo = mlop.tile([P, GBV * c_lo * hid], dt.float8e4, tag="mlo")
                nc.gpsimd.dma_gather(
                    out_ap=mlo[:, :gb * c_lo * hid]
                        .rearrange("p (c f) -> p c f", c=gb * c_lo),
                    in_ap=tlo_d[:, :],
                    idxs_ap=idxlo_sb[:, g0 * c_lo * 8:(g0 + gb) * c_lo * 8],
                    num_idxs=gb * c_lo * P,
                    num_idxs_reg=gb * c_lo * P,
                    elem_size=hid, single_packet=False, queue_num=next_q())
                mlo_t[g] = mlo

            def emit_hi(g):
                g0 = g * GBV
                gb = min(GBV, nblk - g0)
                st = stp.tile([P, GBV * c_tot * P], dt.float8e4, tag="st")
                nc.sync.dma_start(
                    out=st[:, :gb * c_tot * P],
                    in_=sth[:, g0 * c_tot * P:(g0 + gb) * c_tot * P])
                mhi = mhip.tile([P, GBV * c_hi * hid], dt.float8e4, tag="mhi")
                nc.gpsimd.dma_gather(
                    out_ap=mhi[:, :gb * c_hi * hid]
                        .rearrange("p (c f) -> p c f", c=gb * c_hi),
                    in_ap=thi_d[:, :],
                    idxs_ap=idxhi_sb[:, g0 * c_hi * 8:(g0 + gb) * c_hi * 8],
                    num_idxs=gb * c_hi * P,
                    num_idxs_reg=gb * c_hi * P,
                    elem_size=hid, single_packet=False, queue_num=next_q())
                st_t[g] = st
                mhi_t[g] = mhi

            for g in range(min(LEAD, ngrp)):
                emit_lo(g)
            if pre_loop is not None:
                pre_loop()
            for g in range(ngrp):
                if g + LEAD < ngrp:
                    emit_lo(g + LEAD)
                emit_hi(g)
                st, mlo, mhi = st_t.pop(g), mlo_t.pop(g), mhi_t.pop(g)
                g0 = g * GBV
                gb = min(GBV, nblk - g0)
                for bi in range(gb):
                    nb = g0 + bi
                    agg = ps_agg.tile([P, hid], dt.float32, tag="agg")
                    for c in range(c_lo):
                        nc.tensor.matmul(
                            out=agg[:, :],
                            lhsT=st[:, (bi * c_tot + c) * P:(bi * c_tot + c + 1) * P],
                            rhs=mlo[:, (bi * c_lo + c) * hid:(bi * c_lo + c + 1) * hid],
                            start=(c == 0), stop=False)
                    for c in range(c_hi):
                        nc.tensor.matmul(
                            out=agg[:, :],
                            lhsT=st[:, (bi * c_tot + c_lo + c) * P:
                                    (bi * c_tot + c_lo + c + 1) * P],
                            rhs=mhi[:, (bi * c_hi + c) * hid:(bi * c_hi + c + 1) * hid],
                            start=False, stop=False)
                    if has_b:
                        # += (1/dinv_dst) outer bias  (undoes the dinv post-scale)
                        nc.tensor.matmul(
                            out=agg[:, :],
                            lhsT=idinv_sb[0:1, nb * P:(nb + 1) * P],
                            rhs=brow_sb[0:1, :],
                            start=False, stop=False)
                    nc.tensor.matmul(
                        out=agg[:, :], lhsT=ident_sb[:, :],
                        rhs=h_sb[:, nb * hid:(nb + 1) * hid],
                        start=False, stop=True)
                    z = zp.tile([P, hid], dt.bfloat16, tag="z")
                    nc.scalar.activation(
                        z[:, :], agg[:, :], mybir.ActivationFunctionType.Relu,
                        scale=dinv_sb[:, nb:nb + 1])
                    z_consumer(nb, z)
                if post_group is not None:
                    post_group(g)

        # ================= layer 1 dense + table AllGathers =================
        for nb in range(nlo_b):
            dense1(nb)
        distribute_lo(tlo1_d)
        for nb in range(nlo_b, nblk):
            dense1(nb)
        distribute_hi_dma()
        distribute_hi_cc(thi1_d)

        # ============ layer 1 aggregate, fused with layer 2 dense ============
        def z1_consumer(nb, z):
            # zT[:, k*nshard + nb*P : ...] = z^T via PE identity transpose
            for k in range(KH):
                pst = ps_tr.tile([P, P], dt.float32, tag="tr")
                nc.tensor.matmul(
                    out=pst[:, :], lhsT=z[:, k * P:(k + 1) * P],
                    rhs=identbf_sb[:, :], start=True, stop=True)
                nc.scalar.activation(
                    zT_sb[:, k * nshard + nb * P:k * nshard + (nb + 1) * P],
                    pst[:, :], mybir.ActivationFunctionType.Copy)
            # layer-2 dense for this block: h2 = fp8(dinv * (z1 @ W2))
            ps2 = ps_mm.tile([P, hid], dt.float32, tag="mm")
            for k in range(KH):
                nc.tensor.matmul(
                    out=ps2[:, :],
                    lhsT=zT_sb[:, k * nshard + nb * P:k * nshard + (nb + 1) * P],
                    rhs=w2_sb[:, k * hid:(k + 1) * hid],
                    start=(k == 0), stop=(k == KH - 1))
            nc.scalar.activation(
                h_sb[:, nb * hid:(nb + 1) * hid], ps2[:, :],
                mybir.ActivationFunctionType.Copy, scale=dinv_sb[:, nb:nb + 1])

        def post1(g):
            if g == g_lo2:
                distribute_lo(tlo2_d)
            if g == ngrp - 1:
                distribute_hi_dma()

        aggregate(tlo1_d, thi1_d, brow1_sb, has_b1, z1_consumer,
                  post_group=post1)

        # ================= layer 2 aggregate + readout =================
        cs = [ps_cs.tile([P, 1], dt.float32, tag=f"cs{h}", name=f"cs{h}")
              for h in range(KH)]

        def colsum(nb, z):
            for h in range(KH):
                nc.tensor.matmul(
                    out=cs[h][:, :], lhsT=z[:, h * P:(h + 1) * P],
                    rhs=mask_sb[:, nb:nb + 1],
                    start=(nb == 0), stop=(nb == nblk - 1))

        aggregate(tlo2_d, thi2_d, brow2_sb, has_b2, colsum,
                  pre_loop=lambda: distribute_hi_cc(thi2_d))

        out_sb = zp.tile([P, KH], dt.float32, tag="out_sb")
        for h in range(KH):
            nc.vector.tensor_copy(out=out_sb[:, h:h + 1], in_=cs[h][:, :])
        nc.sync.dma_start(out=out[:, :], in_=out_sb[:, :])

    nc.compile()
    return nc


# --------------------------------------------------------------------------- #
# Entry point
# --------------------------------------------------------------------------- #

_CACHE = {}


def _run(x, edge_index, W1, b1, W2, b2, trace=False):
    from concourse.bass_utils import run_bass_kernel_spmd

    in_maps, meta = _preprocess(x, edge_index, W1, b1, W2, b2)
    key = (meta["nblk"], meta["nlo_b"], meta["c_lo"], meta["c_hi"],
           meta["in_dim"], meta["hid"], meta["has_b1"], meta["has_b2"])
    if key not in _CACHE:
        _CACHE[key] = _build_nc(*key)
    nc = _CACHE[key]
    res = run_bass_kernel_spmd(
        nc, in_maps, core_ids=list(range(N_CORES)), trace=trace)
    parts = [r["partial"] for r in res.results]  # each [P, KH] f32
    colsum = np.sum(np.stack(parts), axis=0)     # [P, KH]
    g = colsum.T.reshape(-1)                     # [hid], g[h*P+p] = colsum[p, h]
    return g / float(meta["n"]), res


def kernel(x, edge_index, W1, b1, W2, b2, Wfc, bfc):
    x = np.asarray(x, dtype=np.float32)
    g, _ = _run(x, edge_index, np.asarray(W1, np.float32), np.asarray(b1, np.float32),
                np.asarray(W2, np.float32), np.asarray(b2, np.float32))
    logits = g.astype(np.float32) @ np.asarray(Wfc, np.float32) + np.asarray(bfc, np.float32)
    return (1.0 / (1.0 + np.exp(-logits))).astype(np.float32)


# revision 6
# speedup vs baseline: 1.1093x; 1.1093x over previous
"""GCN (2-layer GCNConv + mean readout + sigmoid head) on 8 Trainium2 NeuronCores.

Strategy (graph/data parallel, dst-sharded):
  - Nodes are permuted (round-robin by in-degree) into NB = n_cores*nblk blocks of
    128 so every block has ~equal in-edge count; each core owns nblk blocks.
  - Per layer: H' = (D^-1/2 Z) @ W computed node-sharded on PE (bf16), written to
    an fp8 feature table, AllGather (Shared output) of the fp8 table, then per
    dst-block: dma_gather of the 256B source rows (table split in two halves so
    row ids fit int16), host-precomputed fp8 one-hot selection matrices streamed
    from HBM via HWDGE, contracted on PE to form segment sums in PSUM.
    Self-loops are added via an fp8 identity-matrix matmul on the local shard.
  - Everything is ordered HI-FIRST so the two AllGathers per table pipeline
    with gather work instead of head-of-line blocking it: dense computes hi
    blocks first (AG-hi fires before AG-lo), hi gathers lead lo gathers by
    HLEAD blocks (they drain while AG-lo is still in flight), and aggregation
    processes hi blocks first so the NEXT layer's AG-hi fires mid-loop.
  - One dst-block per gather + deep hi-side msg buffering keeps all 4 SWDGE
    queues draining instead of coupling gather issue to PE progress.
  - Layer-2 work is fused into layer-1 aggregation: each z1 block is
    PE-transposed (identity matmul) into zT and immediately pushed through the
    layer-2 dense matmul, so the layer-2 AllGathers hide under aggregation.
  - dinv factors are separable: dinv_src is folded into the table rows,
    dinv_dst + relu are applied post-aggregation in ONE ScalarE activation.
  - Readout: per-block column sums via matmul against a pad-mask vector,
    accumulated in PSUM; final cross-core reduce + fc + sigmoid on host.
"""

import math

import numpy as np
import ml_dtypes

BF16 = ml_dtypes.bfloat16
FP8 = ml_dtypes.float8_e4m3  # == mybir.dt.float8e4 (IEEE, max +-240)

# Problem constants (hardcoded per contract; kernel.py must be self-contained).
N = 50000
E = 800000
IN_DIM = 512
HID = 256
N_CORES = 8
P = 128
HLEAD = 14  # hi gathers lead lo gathers by this many blocks


def _wrap_idx(flat):
    """[L] int -> [128, L/16] int16 in the SWDGE wrapped layout."""
    L = len(flat)
    assert L % 16 == 0
    w = flat.reshape(L // 16, 16).T  # value i at [i%16, i//16]
    return np.ascontiguousarray(np.tile(w, (8, 1)).astype(np.int16))


# --------------------------------------------------------------------------- #
# Host-side preprocessing
# --------------------------------------------------------------------------- #

def _preprocess(x, edge_index, W1, b1, W2, b2):
    n, in_dim = x.shape
    hid = W1.shape[1]
    src = np.asarray(edge_index[0], dtype=np.int64)
    dst = np.asarray(edge_index[1], dtype=np.int64)

    deg_in = np.bincount(dst, minlength=n)
    deg = deg_in.astype(np.float64) + 1.0  # + self loop
    dinv = (1.0 / np.sqrt(deg)).astype(np.float32)

    nblk = math.ceil(n / (N_CORES * P))          # blocks per core
    NB = N_CORES * nblk                          # total blocks
    npad = NB * P
    nshard = nblk * P
    nlo_b = (nblk + 1) // 2                      # lo blocks per core
    nhi_b = nblk - nlo_b
    nlo, nhi = nlo_b * P, nhi_b * P
    assert N_CORES * nlo <= 32767, "lo table must fit int16"

    # Balance blocks: deal nodes round-robin across blocks in desc in-degree
    # order -> every block gets ~equal total in-degree.
    order = np.argsort(-deg_in, kind="stable")
    i = np.arange(n)
    new_id = np.empty(n, dtype=np.int64)
    new_id[order] = (i % NB) * P + (i // NB)

    # Edge arrays in permuted space, sorted by (dst block, src half).
    # Source "half" = within-shard block position (hi = last nhi_b blocks of
    # each core's shard); the hi table is AllGathered first, so hi gathers
    # overlap with the lo AllGather.
    s_new = new_id[src]
    d_new = new_id[dst]
    blk_id = d_new // P
    s_core = (s_new // P) // nblk
    s_blocal = (s_new // P) % nblk
    s_pos = s_blocal * P + (s_new % P)
    is_hi = (s_blocal >= nlo_b).astype(np.int64)
    s_lo_id = s_core * nlo + s_pos
    s_hi_id = s_core * nhi + (s_pos - nlo)
    skey = blk_id * 2 + is_hi
    eorder = np.argsort(skey, kind="stable")
    s_lo_id = s_lo_id[eorder]
    s_hi_id = s_hi_id[eorder]
    d_new = d_new[eorder]
    key_sorted = skey[eorder]

    cnt = np.bincount(key_sorted, minlength=2 * NB).reshape(NB, 2)
    c_lo = max(1, int(math.ceil(cnt[:, 0].max() / P)))
    c_hi = max(1, int(math.ceil(cnt[:, 1].max() / P)))
    c_tot = c_lo + c_hi

    # Per-(block, half) padded slots.
    idx_lo = np.zeros((NB, c_lo * P), dtype=np.int64)
    idx_hi = np.zeros((NB, c_hi * P), dtype=np.int64)
    dst_arr = np.full((NB, c_tot * P), -1.0, dtype=np.float32)

    starts = np.zeros(2 * NB + 1, dtype=np.int64)
    np.cumsum(cnt.reshape(-1), out=starts[1:])
    pos = np.arange(len(s_new)) - starts[key_sorted]
    lo_m = key_sorted % 2 == 0
    hi_m = ~lo_m
    b_lo, b_hi = key_sorted[lo_m] // 2, key_sorted[hi_m] // 2
    idx_lo[b_lo, pos[lo_m]] = s_lo_id[lo_m]
    idx_hi[b_hi, pos[hi_m]] = s_hi_id[hi_m]
    dst_arr[b_lo, pos[lo_m]] = (d_new[lo_m] % P).astype(np.float32)
    dst_arr[b_hi, c_lo * P + pos[hi_m]] = (d_new[hi_m] % P).astype(np.float32)

    dst_arr = dst_arr.reshape(NB, c_tot, P)

    # Host-built one-hot scatter matrices: st[eslot, (blk, c, dstq)] in fp8.
    # (dst_arr == -1 padding rows become all-zero one-hot rows.)
    onehot = (dst_arr[:, :, :, None] ==
              np.arange(P, dtype=np.float32)[None, None, None, :])
    # [NB, c_tot, eslot, dstq] -> [eslot, NB, c_tot, dstq]
    st_all = np.ascontiguousarray(
        onehot.transpose(2, 0, 1, 3).astype(FP8))

    # x' = dinv * x, permuted, padded, per-core transposed, bf16.
    xp = np.zeros((npad, in_dim), dtype=np.float32)
    xp[new_id] = x * dinv[:, None]

    dinv_pad = np.zeros(npad, dtype=np.float32)
    dinv_pad[new_id] = dinv
    inv_dinv_pad = np.zeros(npad, dtype=np.float32)
    inv_dinv_pad[new_id] = 1.0 / dinv
    mask_pad = np.zeros(npad, dtype=np.float32)
    mask_pad[new_id] = 1.0

    ident = np.eye(P, dtype=np.float32)
    has_b1 = bool(np.any(np.asarray(b1) != 0.0))
    has_b2 = bool(np.any(np.asarray(b2) != 0.0))

    common = {
        "w1": np.ascontiguousarray(W1.astype(BF16)),
        "w2": np.ascontiguousarray(W2.astype(BF16)),
        "brow1": np.ascontiguousarray(np.asarray(b1, np.float32).reshape(1, hid)),
        "brow2": np.ascontiguousarray(np.asarray(b2, np.float32).reshape(1, hid)),
        "ident": np.ascontiguousarray(ident.astype(FP8)),
        "identbf": np.ascontiguousarray(ident.astype(BF16)),
    }

    in_maps = []
    for c in range(N_CORES):
        lo_b, hi_b = c * nblk, (c + 1) * nblk
        lo_n, hi_n = c * nshard, (c + 1) * nshard
        m = dict(common)
        m["xT"] = np.ascontiguousarray(xp[lo_n:hi_n].T.astype(BF16))
        m["idxlo"] = _wrap_idx(idx_lo[lo_b:hi_b].reshape(-1))
        m["idxhi"] = _wrap_idx(idx_hi[lo_b:hi_b].reshape(-1))
        # [eslot, nblk, c_tot, dstq] -> [P, nblk*c_tot*P]
        m["sth"] = np.ascontiguousarray(
            st_all[:, lo_b:hi_b].reshape(P, nblk * c_tot * P))
        m["dinv"] = np.ascontiguousarray(
            dinv_pad[lo_n:hi_n].reshape(nblk, P).T.astype(np.float32))
        m["idinv"] = np.ascontiguousarray(
            inv_dinv_pad[lo_n:hi_n].reshape(1, nshard).astype(np.float32))
        m["maskc"] = np.ascontiguousarray(
            mask_pad[lo_n:hi_n].reshape(nblk, P).T.astype(BF16))
        in_maps.append(m)

    meta = dict(nblk=nblk, nlo_b=nlo_b, c_lo=c_lo, c_hi=c_hi, in_dim=in_dim,
                hid=hid, n=n, has_b1=has_b1, has_b2=has_b2)
    return in_maps, meta


# --------------------------------------------------------------------------- #
# Device program
# --------------------------------------------------------------------------- #

def _build_nc(nblk, nlo_b, c_lo, c_hi, in_dim, hid, has_b1=False, has_b2=False):
    from contextlib import ExitStack

    from concourse import bass, mybir, bacc
    import concourse.tile as tile

    dt = mybir.dt
    nshard = nblk * P
    npad = N_CORES * nshard
    nhi_b = nblk - nlo_b
    nlo, nhi = nlo_b * P, nhi_b * P
    KIN = in_dim // P
    KH = hid // P
    c_tot = c_lo + c_hi
    # hi blocks first: their dense outputs feed the hi AllGather, which must
    # precede the lo AllGather on the (serial) cc stream.
    blk_order = list(range(nlo_b, nblk)) + list(range(nlo_b))

    nc = bacc.Bacc(None, target_bir_lowering=False, num_devices=N_CORES,
                   num_swdge_queues=4)

    xT = nc.dram_tensor("xT", [in_dim, nshard], dt.bfloat16, kind="ExternalInput")
    w1 = nc.dram_tensor("w1", [in_dim, hid], dt.bfloat16, kind="ExternalInput")
    w2 = nc.dram_tensor("w2", [hid, hid], dt.bfloat16, kind="ExternalInput")
    brow1 = nc.dram_tensor("brow1", [1, hid], dt.float32, kind="ExternalInput")
    brow2 = nc.dram_tensor("brow2", [1, hid], dt.float32, kind="ExternalInput")
    ident = nc.dram_tensor("ident", [P, P], dt.float8e4, kind="ExternalInput")
    identbf = nc.dram_tensor("identbf", [P, P], dt.bfloat16, kind="ExternalInput")
    idxlo = nc.dram_tensor("idxlo", [P, nblk * c_lo * 8], dt.int16, kind="ExternalInput")
    idxhi = nc.dram_tensor("idxhi", [P, nblk * c_hi * 8], dt.int16, kind="ExternalInput")
    sth = nc.dram_tensor("sth", [P, nblk * c_tot * P], dt.float8e4, kind="ExternalInput")
    dinv = nc.dram_tensor("dinv", [P, nblk], dt.float32, kind="ExternalInput")
    idinv = nc.dram_tensor("idinv", [1, nshard], dt.float32, kind="ExternalInput")
    maskc = nc.dram_tensor("maskc", [P, nblk], dt.bfloat16, kind="ExternalInput")
    out = nc.dram_tensor("partial", [P, KH], dt.float32, kind="ExternalOutput")

    with tile.TileContext(nc) as tc, ExitStack() as ctx:
        const = ctx.enter_context(tc.tile_pool(name="const", bufs=1))
        persist = ctx.enter_context(tc.tile_pool(name="persist", bufs=1))
        lhsp = ctx.enter_context(tc.tile_pool(name="lhsp", bufs=1))
        mlop = ctx.enter_context(tc.tile_pool(name="mlop", bufs=4))
        mhip = ctx.enter_context(tc.tile_pool(name="mhip", bufs=HLEAD + 2))
        stp = ctx.enter_context(tc.tile_pool(name="stp", bufs=6))
        zp = ctx.enter_context(tc.tile_pool(name="zp", bufs=4))
        ps_mm = ctx.enter_context(tc.tile_pool(name="ps_mm", bufs=2, space="PSUM"))
        ps_agg = ctx.enter_context(tc.tile_pool(name="ps_agg", bufs=2, space="PSUM"))
        ps_tr = ctx.enter_context(tc.tile_pool(name="ps_tr", bufs=2, space="PSUM"))
        ps_cs = ctx.enter_context(tc.tile_pool(name="ps_cs", bufs=1, space="PSUM"))
        dram = ctx.enter_context(tc.tile_pool(name="dram", bufs=1, space="DRAM"))

        # ---- persistent / constant tiles ----
        w1_sb = const.tile([P, KIN * hid], dt.bfloat16, tag="w1_sb")
        w2_sb = const.tile([P, KH * hid], dt.bfloat16, tag="w2_sb")
        brow1_sb = const.tile([1, hid], dt.float32, tag="brow1_sb")
        brow2_sb = const.tile([1, hid], dt.float32, tag="brow2_sb")
        ident_sb = const.tile([P, P], dt.float8e4, tag="ident_sb")
        identbf_sb = const.tile([P, P], dt.bfloat16, tag="identbf_sb")
        idxlo_sb = const.tile([P, nblk * c_lo * 8], dt.int16, tag="idxlo_sb")
        idxhi_sb = const.tile([P, nblk * c_hi * 8], dt.int16, tag="idxhi_sb")
        dinv_sb = const.tile([P, nblk], dt.float32, tag="dinv_sb")
        mask_sb = const.tile([P, nblk], dt.bfloat16, tag="mask_sb")
        if has_b1 or has_b2:
            idinv_sb = const.tile([1, nshard], dt.float32, tag="idinv_sb")

        zT_sb = persist.tile([P, KH * nshard], dt.bfloat16, tag="zT_sb")
        h_sb = persist.tile([P, nblk * hid], dt.float8e4, tag="h_sb")

        hshard_lo_d = dram.tile([nlo, hid], dt.float8e4, tag="hshard_lo_d")
        hshard_hi_d = dram.tile([nhi, hid], dt.float8e4, tag="hshard_hi_d")
        # Shared DRAM (collective outputs) may be written by a single inst
        # each -> one table per (layer, half).
        tlo1_d = dram.tile([N_CORES * nlo, hid], dt.float8e4, tag="tlo1_d",
                           addr_space="Shared")
        thi1_d = dram.tile([N_CORES * nhi, hid], dt.float8e4, tag="thi1_d",
                           addr_space="Shared")
        tlo2_d = dram.tile([N_CORES * nlo, hid], dt.float8e4, tag="tlo2_d",
                           addr_space="Shared")
        thi2_d = dram.tile([N_CORES * nhi, hid], dt.float8e4, tag="thi2_d",
                           addr_space="Shared")

        # ---- constant loads (w1 + x first: they gate dense1; hi x first) ----
        nc.sync.dma_start(
            out=w1_sb[:, :].rearrange("p (k f) -> p k f", k=KIN),
            in_=w1[:, :].rearrange("(k p) f -> p k f", p=P))
        xfull = lhsp.tile([P, KIN * nshard], dt.bfloat16, tag="xfull")
        nc.sync.dma_start(
            out=xfull[:, :].rearrange("p (k n) -> p k n", k=KIN)[:, :, nlo:],
            in_=xT[:, nlo:].rearrange("(k p) n -> p k n", p=P))
        nc.sync.dma_start(
            out=xfull[:, :].rearrange("p (k n) -> p k n", k=KIN)[:, :, :nlo],
            in_=xT[:, :nlo].rearrange("(k p) n -> p k n", p=P))
        nc.sync.dma_start(
            out=w2_sb[:, :].rearrange("p (k f) -> p k f", k=KH),
            in_=w2[:, :].rearrange("(k p) f -> p k f", p=P))
        nc.sync.dma_start(out=brow1_sb[:, :], in_=brow1[:, :])
        nc.sync.dma_start(out=brow2_sb[:, :], in_=brow2[:, :])
        nc.sync.dma_start(out=ident_sb[:, :], in_=ident[:, :])
        nc.sync.dma_start(out=identbf_sb[:, :], in_=identbf[:, :])
        nc.sync.dma_start(out=idxhi_sb[:, :], in_=idxhi[:, :])
        nc.sync.dma_start(out=idxlo_sb[:, :], in_=idxlo[:, :])
        nc.sync.dma_start(out=dinv_sb[:, :], in_=dinv[:, :])
        nc.sync.dma_start(out=mask_sb[:, :], in_=maskc[:, :])
        if has_b1 or has_b2:
            nc.sync.dma_start(out=idinv_sb[:, :], in_=idinv[:, :])

        def dense1(nb):
            """h_sb[:, nb*hid:...] = fp8(x' @ W1) for one block."""
            ps = ps_mm.tile([P, hid], dt.float32, tag="mm")
            for k in range(KIN):
                nc.tensor.matmul(
                    out=ps[:, :],
                    lhsT=xfull[:, k * nshard + nb * P:k * nshard + (nb + 1) * P],
                    rhs=w1_sb[:, k * hid:(k + 1) * hid],
                    start=(k == 0), stop=(k == KIN - 1))
            nc.scalar.activation(
                h_sb[:, nb * hid:(nb + 1) * hid], ps[:, :],
                mybir.ActivationFunctionType.Copy)

        def distribute_hi(table_d):
            nc.scalar.dma_start(
                out=hshard_hi_d[:, :].rearrange("(nb p) f -> p nb f", p=P),
                in_=h_sb[:, nlo_b * hid:].rearrange("p (nb f) -> p nb f",
                                                    nb=nhi_b))
            nc.gpsimd.collective_compute(
                "AllGather", mybir.AluOpType.bypass,
                replica_groups=[list(range(N_CORES))],
                ins=[hshard_hi_d[:, :].opt()],
                outs=[table_d[:, :].opt()])

        def distribute_lo(table_d):
            nc.scalar.dma_start(
                out=hshard_lo_d[:, :].rearrange("(nb p) f -> p nb f", p=P),
                in_=h_sb[:, :nlo_b * hid].rearrange("p (nb f) -> p nb f",
                                                    nb=nlo_b))
            nc.gpsimd.collective_compute(
                "AllGather", mybir.AluOpType.bypass,
                replica_groups=[list(range(N_CORES))],
                ins=[hshard_lo_d[:, :].opt()],
                outs=[table_d[:, :].opt()])

        def aggregate(tlo_d, thi_d, brow_sb, has_b, z_consumer, post_idx=None):
            """Per dst block (hi blocks first): gather + one-hot contract.

            Hi gathers lead lo gathers by HLEAD blocks: they only need the hi
            table (AllGathered first) and drain while the lo AllGather is
            still in flight.
            """
            qi = [0]

            def next_q():
                q = qi[0] % 4
                qi[0] += 1
                return q

            mlo_t, mhi_t, st_t = {}, {}, {}

            def emit_hi(nb):
                mhi = mhip.tile([P, c_hi * hid], dt.float8e4, tag="mhi")
                nc.gpsimd.dma_gather(
                    out_ap=mhi[:, :].rearrange("p (c f) -> p c f", c=c_hi),
                    in_ap=thi_d[:, :],
                    idxs_ap=idxhi_sb[:, nb * c_hi * 8:(nb + 1) * c_hi * 8],
                    num_idxs=c_hi * P,
                    num_idxs_reg=c_hi * P,
                    elem_size=hid, single_packet=False, queue_num=next_q())
                mhi_t[nb] = mhi

            def emit_lo(nb):
                st = stp.tile([P, c_tot * P], dt.float8e4, tag="st")
                nc.sync.dma_start(
                    out=st[:, :],
                    in_=sth[:, nb * c_tot * P:(nb + 1) * c_tot * P])
                mlo = mlop.tile([P, c_lo * hid], dt.float8e4, tag="mlo")
                nc.gpsimd.dma_gather(
                    out_ap=mlo[:, :].rearrange("p (c f) -> p c f", c=c_lo),
                    in_ap=tlo_d[:, :],
                    idxs_ap=idxlo_sb[:, nb * c_lo * 8:(nb + 1) * c_lo * 8],
                    num_idxs=c_lo * P,
                    num_idxs_reg=c_lo * P,
                    elem_size=hid, single_packet=False, queue_num=next_q())
                st_t[nb] = st
                mlo_t[nb] = mlo

            for j in range(min(HLEAD, nblk)):
                emit_hi(blk_order[j])
            for i, nb in enumerate(blk_order):
                if i + HLEAD < nblk:
                    emit_hi(blk_order[i + HLEAD])
                emit_lo(nb)
                st, mlo, mhi = st_t.pop(nb), mlo_t.pop(nb), mhi_t.pop(nb)
                agg = ps_agg.tile([P, hid], dt.float32, tag="agg")
                for c in range(c_hi):
                    nc.tensor.matmul(
                        out=agg[:, :],
                        lhsT=st[:, (c_lo + c) * P:(c_lo + c + 1) * P],
                        rhs=mhi[:, c * hid:(c + 1) * hid],
                        start=(c == 0), stop=False)
                for c in range(c_lo):
                    nc.tensor.matmul(
                        out=agg[:, :],
                        lhsT=st[:, c * P:(c + 1) * P],
                        rhs=mlo[:, c * hid:(c + 1) * hid],
                        start=False, stop=False)
                if has_b:
                    # += (1/dinv_dst) outer bias  (undoes the dinv post-scale)
                    nc.tensor.matmul(
                        out=agg[:, :],
                        lhsT=idinv_sb[0:1, nb * P:(nb + 1) * P],
                        rhs=brow_sb[0:1, :],
                        start=False, stop=False)
                nc.tensor.matmul(
                    out=agg[:, :], lhsT=ident_sb[:, :],
                    rhs=h_sb[:, nb * hid:(nb + 1) * hid],
                    start=False, stop=True)
                z = zp.tile([P, hid], dt.bfloat16, tag="z")
                nc.scalar.activation(
                    z[:, :], agg[:, :], mybir.ActivationFunctionType.Relu,
                    scale=dinv_sb[:, nb:nb + 1])
                z_consumer(i, nb, z)
                if post_idx is not None:
                    post_idx(i)

        # ================= layer 1 dense + table AllGathers =================
        for nb in blk_order:
            dense1(nb)
            if nb == blk_order[nhi_b - 1]:
                distribute_hi(thi1_d)
        distribute_lo(tlo1_d)

        # ============ layer 1 aggregate, fused with layer 2 dense ============
        def z1_consumer(i, nb, z):
            # zT[:, k*nshard + nb*P : ...] = z^T via PE identity transpose
            for k in range(KH):
                pst = ps_tr.tile([P, P], dt.float32, tag="tr")
                nc.tensor.matmul(
                    out=pst[:, :], lhsT=z[:, k * P:(k + 1) * P],
                    rhs=identbf_sb[:, :], start=True, stop=True)
                nc.scalar.activation(
                    zT_sb[:, k * nshard + nb * P:k * nshard + (nb + 1) * P],
                    pst[:, :], mybir.ActivationFunctionType.Copy)
            # layer-2 dense for this block: h2 = fp8(dinv * (z1 @ W2))
            ps2 = ps_mm.tile([P, hid], dt.float32, tag="mm")
            for k in range(KH):
                nc.tensor.matmul(
                    out=ps2[:, :],
                    lhsT=zT_sb[:, k * nshard + nb * P:k * nshard + (nb + 1) * P],
                    rhs=w2_sb[:, k * hid:(k + 1) * hid],
                    start=(k == 0), stop=(k == KH - 1))
            nc.scalar.activation(
                h_sb[:, nb * hid:(nb + 1) * hid], ps2[:, :],
                mybir.ActivationFunctionType.Copy, scale=dinv_sb[:, nb:nb + 1])

        def post1(i):
            # hi blocks come first in blk_order, so after index nhi_b-1 the
            # whole hi half of h2 exists and its AllGather can fire.
            if i == nhi_b - 1:
                distribute_hi(thi2_d)
            if i == nblk - 1:
                distribute_lo(tlo2_d)

        aggregate(tlo1_d, thi1_d, brow1_sb, has_b1, z1_consumer,
                  post_idx=post1)

        # ================= layer 2 aggregate + readout =================
        cs = ps_cs.tile([P, KH], dt.float32, tag="cs", name="cs")

        def colsum(i, nb, z):
            for h in range(KH):
                nc.tensor.matmul(
                    out=cs[:, h:h + 1], lhsT=z[:, h * P:(h + 1) * P],
                    rhs=mask_sb[:, nb:nb + 1],
                    start=(i == 0), stop=(i == nblk - 1))

        aggregate(tlo2_d, thi2_d, brow2_sb, has_b2, colsum)

        out_sb = zp.tile([P, KH], dt.float32, tag="out_sb")
        nc.vector.tensor_copy(out=out_sb[:, :], in_=cs[:, :])
        nc.sync.dma_start(out=out[:, :], in_=out_sb[:, :])

    nc.compile()
    return nc


# --------------------------------------------------------------------------- #
# Entry point
# --------------------------------------------------------------------------- #

_CACHE = {}


def _run(x, edge_index, W1, b1, W2, b2, trace=False):
    from concourse.bass_utils import run_bass_kernel_spmd

    in_maps, meta = _preprocess(x, edge_index, W1, b1, W2, b2)
    key = (meta["nblk"], meta["nlo_b"], meta["c_lo"], meta["c_hi"],
           meta["in_dim"], meta["hid"], meta["has_b1"], meta["has_b2"])
    if key not in _CACHE:
        _CACHE[key] = _build_nc(*key)
    nc = _CACHE[key]
    res = run_bass_kernel_spmd(
        nc, in_maps, core_ids=list(range(N_CORES)), trace=trace)
    parts = [r["partial"] for r in res.results]  # each [P, KH] f32
    colsum = np.sum(np.stack(parts), axis=0)     # [P, KH]
    g = colsum.T.reshape(-1)                     # [hid], g[h*P+p] = colsum[p, h]
    return g / float(meta["n"]), res


def kernel(x, edge_index, W1, b1, W2, b2, Wfc, bfc):
    x = np.asarray(x, dtype=np.float32)
    g, _ = _run(x, edge_index, np.asarray(W1, np.float32), np.asarray(b1, np.float32),
                np.asarray(W2, np.float32), np.asarray(b2, np.float32))
    logits = g.astype(np.float32) @ np.asarray(Wfc, np.float32) + np.asarray(bfc, np.float32)
    return (1.0 / (1.0 + np.exp(-logits))).astype(np.float32)
